# revision 8
# baseline (speedup 1.0000x reference)
"""DiffAttnV2-like fused kernel for Trainium2 (8 NeuronCores).

Sharding: core = 4*b + g  (b = batch 0..1, g = head-group 0..3, 4 heads each).
Each core computes its 4 output heads' attention and a partial out = y_g @ Wo_g;
host sums the 4 partials per batch.

Per-core dataflow (float32r matmuls - full PE rate, ~1.5e-4 rel rounding):
  4 phases over t-columns (512 each):
    projections into transposed layouts (qT/kT [d,t]; v natural [t,d]; lamT)
    causal attention in sT=[tk,tq] layout; ACT exp evacuates PSUM;
    denominator via ones-column matmul; normalize/combine via K=1 broadcast
    matmuls; partial output projection streamed per 512-col group.
"""
import sys
sys.path.insert(0, "/opt/trn_rl_repo")
from contextlib import ExitStack

import numpy as np

from concourse import bacc, mybir, tile
from concourse.bass_utils import run_bass_kernel_spmd

B, T, D, H = 2, 2048, 2048, 16
HPC = 4               # heads per core
NC = 8                # cores
NDC = D // 128        # 16 contraction chunks
NPH = 4               # t-phases
PT = T // NPH         # 512 t-cols per phase
SCALE = 1.0 / float(np.sqrt(D // H))

f32 = mybir.dt.float32
f32r = mybir.dt.float32r
EXP = mybir.ActivationFunctionType.Exp
SIG = mybir.ActivationFunctionType.Sigmoid

_CACHE = {}


def _build():
    nc = bacc.Bacc("TRN2", target_bir_lowering=False, debug=False)
    xT = nc.dram_tensor("xT", [D, T], f32r, kind="ExternalInput").ap()
    wq1 = nc.dram_tensor("wq1", [D, 512], f32r, kind="ExternalInput").ap()
    wq2 = nc.dram_tensor("wq2", [D, 512], f32r, kind="ExternalInput").ap()
    wk = nc.dram_tensor("wk", [D, 512], f32r, kind="ExternalInput").ap()
    wv = nc.dram_tensor("wv", [D, 512], f32r, kind="ExternalInput").ap()
    wlam = nc.dram_tensor("wlam", [D, HPC], f32r, kind="ExternalInput").ap()
    wo = nc.dram_tensor("wo", [512, D], f32r, kind="ExternalInput").ap()
    mstrip = nc.dram_tensor("mstrip", [128, 896], f32r, kind="ExternalInput").ap()
    selin = nc.dram_tensor("selin", [HPC, 512], f32r, kind="ExternalInput").ap()
    out = nc.dram_tensor("out", [T, D], f32, kind="ExternalOutput").ap()

    with tile.TileContext(nc) as tc, ExitStack() as ctx:
        ctx.enter_context(nc.allow_low_precision(reason="fp32r matmul pipeline"))
        persist = ctx.enter_context(tc.tile_pool(name="persist", bufs=1))
        xpool = ctx.enter_context(tc.tile_pool(name="xpool", bufs=1))
        qpool = ctx.enter_context(tc.tile_pool(name="qpool", bufs=1))
        wpool = ctx.enter_context(tc.tile_pool(name="wpool", bufs=2))
        epool = ctx.enter_context(tc.tile_pool(name="epool", bufs=2))
        cpool = ctx.enter_context(tc.tile_pool(name="cpool", bufs=1))
        opool = ctx.enter_context(tc.tile_pool(name="opool", bufs=2))
        # PSUM: mm2 (2 banks x2) + y (1 bank x2) + den (1 bank x2) = 8 banks
        ppmm = ctx.enter_context(tc.tile_pool(name="ppmm", bufs=2, space="PSUM"))
        ppy = ctx.enter_context(tc.tile_pool(name="ppy", bufs=2, space="PSUM"))
        ppden = ctx.enter_context(tc.tile_pool(name="ppden", bufs=2, space="PSUM"))

        # persistent tensors
        kT = persist.tile([128, HPC, T], f32r)          # 32KB
        vn = persist.tile([128, 2, NDC, 2, 128], f32r)  # 32KB [tk,(pair,tkc,j),d]
        ms = persist.tile([128, 896], f32r)             # 3.5KB
        nc.sync.dma_start(out=ms[:], in_=mstrip[:])
        sel = persist.tile([HPC, HPC, 128], f32r)       # head-row selectors
        nc.sync.dma_start(out=sel.rearrange("p a b -> p (a b)"), in_=selin[:])
        ones_col_f = persist.tile([128, 1], f32)
        nc.vector.memset(ones_col_f[:], 1.0)
        ones_col = persist.tile([128, 1], f32r)
        nc.vector.tensor_copy(ones_col[:], ones_col_f[:])
        ones_row_f = persist.tile([1, 128], f32)
        nc.vector.memset(ones_row_f[:], 1.0)
        ones_row = persist.tile([1, 128], f32r)
        nc.vector.tensor_copy(ones_row[:], ones_row_f[:])

        for ph in range(NPH):
            t0 = PT * ph
            # ---- x^T slice for this phase: [128, dc, 512] ----
            xTh = xpool.tile([128, NDC, PT], f32r, name=f"xTh{ph}", tag="xTh")
            for dc in range(NDC):
                nc.sync.dma_start(out=xTh[:, dc],
                                  in_=xT[128 * dc:128 * (dc + 1), t0:t0 + PT])

            # ---- q projections (8 q-heads: 0..3 from wq1, 4..7 from wq2) ----
            qTh = qpool.tile([128, 8, PT], f32r, name=f"qTh{ph}", tag="qTh")
            for qh in range(8):
                wsrc = wq1 if qh < 4 else wq2
                hl = qh % 4
                wt = wpool.tile([128, NDC, 128], f32r, name=f"wq{ph}_{qh}", tag="wq")
                for dc in range(NDC):
                    nc.sync.dma_start(
                        out=wt[:, dc],
                        in_=wsrc[128 * dc:128 * (dc + 1), 128 * hl:128 * (hl + 1)])
                ps = ppmm.tile([128, PT], f32, name=f"psq{ph}_{qh}", tag="mm2")
                for dc in range(NDC):
                    nc.tensor.matmul(ps[:], wt[:, dc], xTh[:, dc],
                                     start=(dc == 0), stop=(dc == NDC - 1))
                nc.vector.tensor_copy(qTh[:, qh], ps[:])

            # ---- k projections (4 k-heads) ----
            for kh in range(HPC):
                wt = wpool.tile([128, NDC, 128], f32r, name=f"wk{ph}_{kh}", tag="wq")
                for dc in range(NDC):
                    nc.sync.dma_start(
                        out=wt[:, dc],
                        in_=wk[128 * dc:128 * (dc + 1), 128 * kh:128 * (kh + 1)])
                ps = ppmm.tile([128, PT], f32, name=f"psk{ph}_{kh}", tag="mm2")
                for dc in range(NDC):
                    nc.tensor.matmul(ps[:], wt[:, dc], xTh[:, dc],
                                     start=(dc == 0), stop=(dc == NDC - 1))
                nc.vector.tensor_copy(kT[:, kh, t0:t0 + PT], ps[:])

            # ---- v projections (2 pairs x 256 cols), natural [tk, d] layout ----
            for pair in range(2):
                wt = wpool.tile([128, NDC, 256], f32r, name=f"wv{ph}_{pair}",
                                tag="wv", bufs=1)
                for dc in range(NDC):
                    nc.sync.dma_start(
                        out=wt[:, dc],
                        in_=wv[128 * dc:128 * (dc + 1), 256 * pair:256 * (pair + 1)])
                for tsub in range(4):
                    tkc = 4 * ph + tsub
                    ps = ppmm.tile([128, 256], f32, name=f"psv{ph}_{pair}_{tsub}",
                                   tag="mm2")
                    for dc in range(NDC):
                        nc.tensor.matmul(
                            ps[:], xTh[:, dc, 128 * tsub:128 * (tsub + 1)],
                            wt[:, dc], start=(dc == 0), stop=(dc == NDC - 1))
                    nc.vector.tensor_copy(
                        vn[:, pair, tkc].rearrange("p a b -> p (a b)"), ps[:])

            # ---- lam projection + sigmoid ----
            wlt = wpool.tile([128, NDC, HPC], f32r, name=f"wl{ph}", tag="wl")
            for dc in range(NDC):
                nc.sync.dma_start(out=wlt[:, dc],
                                  in_=wlam[128 * dc:128 * (dc + 1), :])
            psl = ppy.tile([HPC, PT], f32, name=f"psl{ph}", tag="y")
            for dc in range(NDC):
                nc.tensor.matmul(psl[:], wlt[:, dc], xTh[:, dc],
                                 start=(dc == 0), stop=(dc == NDC - 1))
            lamS = cpool.tile([HPC, PT], f32r, name=f"lam{ph}", tag="lam", bufs=2)
            nc.scalar.activation(lamS[:], psl[:], SIG)

            # ---- attention for tq-group [t0, t0+512), 4 head-pairs ----
            ntk = 4 * (ph + 1)
            yh = qpool.tile([128, HPC, PT], f32r, name=f"yh{ph}", tag="yh")
            for hl in range(HPC):
                y_ps = {}
                rden = cpool.tile([1, 2, PT], f32r, name=f"rden{ph}_{hl}", tag="rden")
                for j, qh in enumerate((hl, 4 + hl)):
                    khl = (hl // 2) if j == 0 else (2 + hl // 2)
                    pair, pj = khl // 2, khl % 2
                    ps_y = ppy.tile([128, PT], f32, name=f"psy{ph}_{hl}_{j}", tag="y")
                    ps_den = ppden.tile([1, PT], f32, name=f"psd{ph}_{hl}_{j}",
                                        tag="den")
                    for bt in range(ntk // 2):
                        ps_s = ppmm.tile([128, 2, PT], f32,
                                         name=f"pss{ph}_{hl}_{j}_{bt}", tag="mm2")
                        for c in range(2):
                            tkc = 2 * bt + c
                            nc.tensor.matmul(
                                ps_s[:, c],
                                kT[:, khl, 128 * tkc:128 * (tkc + 1)],
                                qTh[:, qh], start=True, stop=True)
                        ex = epool.tile([128, 2, PT], f32r,
                                        name=f"ex{ph}_{hl}_{j}_{bt}", tag="ex")
                        nc.scalar.activation(ex[:], ps_s[:], EXP, scale=SCALE)
                        for c in range(2):
                            tkc = 2 * bt + c
                            o = 128 * tkc - t0
                            if o >= 0:   # diagonal tile -> 0/1 mask
                                nc.vector.tensor_mul(ex[:, c], ex[:, c],
                                                     ms[:, 384 - o:896 - o])
                            nc.tensor.matmul(ps_den[:], ones_col[:], ex[:, c],
                                             start=(tkc == 0), stop=(tkc == ntk - 1))
                            nc.tensor.matmul(ps_y[:], vn[:, pair, tkc, pj], ex[:, c],
                                             start=(tkc == 0), stop=(tkc == ntk - 1))
                    y_ps[j] = ps_y
                    nc.vector.reciprocal(rden[:, j], ps_den[:])

                # combine y_h = y1*r1 - lam_h*(r2*y2)
                ps_b = ppmm.tile([128, 2, PT], f32, name=f"psb{ph}_{hl}", tag="mm2")
                nc.tensor.matmul(ps_b[:, 0], ones_row[:], rden[:, 0],
                                 start=True, stop=True)
                nc.tensor.matmul(ps_b[:, 1], ones_row[:], rden[:, 1],
                                 start=True, stop=True)
                ps_lam = ppmm.tile([128, PT], f32, name=f"pslam{ph}_{hl}", tag="mm2")
                nc.tensor.matmul(ps_lam[:], sel[:, hl], lamS[:],
                                 start=True, stop=True)
                rB = cpool.tile([128, 2, PT], f32, name=f"rB{ph}_{hl}", tag="rB")
                nc.vector.tensor_copy(rB[:], ps_b[:])
                t1 = cpool.tile([128, PT], f32, name=f"t1{ph}_{hl}", tag="t1")
                nc.vector.tensor_mul(t1[:], y_ps[0][:], rB[:, 0])
                t2 = cpool.tile([128, PT], f32, name=f"t2{ph}_{hl}", tag="t2")
                nc.vector.tensor_mul(t2[:], y_ps[1][:], rB[:, 1])
                nc.vector.tensor_mul(t2[:], t2[:], ps_lam[:])
                nc.vector.tensor_sub(yh[:, hl], t1[:], t2[:])

            # ---- Wo partial: out[t0:t0+512, :] = sum_h yh^T_h @ wo_h ----
            for dout in range(4):
                wo4 = wpool.tile([128, HPC, 512], f32r, name=f"wo{ph}_{dout}",
                                 tag="wo4", bufs=2)
                for hl in range(HPC):
                    nc.sync.dma_start(
                        out=wo4[:, hl],
                        in_=wo[128 * hl:128 * (hl + 1), 512 * dout:512 * (dout + 1)])
                for tsub in range(4):
                    ps_o = ppmm.tile([128, 512], f32, name=f"pso{ph}_{dout}_{tsub}",
                                     tag="mm2")
                    for hl in range(HPC):
                        nc.tensor.matmul(
                            ps_o[:], yh[:, hl, 128 * tsub:128 * (tsub + 1)],
                            wo4[:, hl], start=(hl == 0), stop=(hl == HPC - 1))
                    ob = opool.tile([128, 512], f32, name=f"ob{ph}_{dout}_{tsub}",
                                    tag="ob")
                    nc.vector.tensor_copy(ob[:], ps_o[:])
                    nc.sync.dma_start(
                        out=out[t0 + 128 * tsub:t0 + 128 * (tsub + 1),
                                512 * dout:512 * (dout + 1)],
                        in_=ob[:])
    nc.compile()
    return nc


def _get_nc():
    if "nc" not in _CACHE:
        _CACHE["nc"] = _build()
    return _CACHE["nc"]


def kernel(x, Wq1, Wq2, Wk, Wv, Wlam, Wo, **_ignored):
    x = np.ascontiguousarray(np.asarray(x, dtype=np.float32))
    Wq1 = np.asarray(Wq1, dtype=np.float32)
    Wq2 = np.asarray(Wq2, dtype=np.float32)
    Wk = np.asarray(Wk, dtype=np.float32)
    Wv = np.asarray(Wv, dtype=np.float32)
    Wlam = np.asarray(Wlam, dtype=np.float32)
    Wo = np.asarray(Wo, dtype=np.float32)

    cc = np.arange(896)[None, :]
    rr = np.arange(128)[:, None]
    mask = (cc >= rr + 384).astype(np.float32)
    selv = np.zeros((HPC, HPC, 128), dtype=np.float32)
    for i in range(HPC):
        selv[i, i, :] = 1.0
    selv = selv.reshape(HPC, 512)

    xTs = [np.ascontiguousarray(x[b].T) for b in range(B)]
    in_maps = []
    for core in range(NC):
        b, g = divmod(core, 4)
        kv_cols = np.r_[256 * g:256 * g + 256, 1024 + 256 * g:1024 + 256 * g + 256]
        in_maps.append({
            "xT": xTs[b],
            "wq1": np.ascontiguousarray(Wq1[:, 512 * g:512 * (g + 1)]),
            "wq2": np.ascontiguousarray(Wq2[:, 512 * g:512 * (g + 1)]),
            "wk": np.ascontiguousarray(Wk[:, kv_cols]),
            "wv": np.ascontiguousarray(Wv[:, kv_cols]),
            "wlam": np.ascontiguousarray(Wlam[:, 4 * g:4 * (g + 1)]),
            "wo": np.ascontiguousarray(Wo[512 * g:512 * (g + 1), :]),
            "mstrip": mask,
            "selin": selv,
        })

    res = run_bass_kernel_spmd(_get_nc(), in_maps, list(range(NC)), **_CACHE.get("run_kwargs", {}))
    _CACHE["last_res"] = res
    out = np.zeros((B, T, D), dtype=np.float32)
    for core in range(NC):
        out[core // 4] += res.results[core]["out"]
    return out


# revision 9
# speedup vs baseline: 1.0403x; 1.0403x over previous
"""DiffAttnV2-like fused kernel for Trainium2 (8 NeuronCores).

Sharding: core = 4*b + g  (b = batch 0..1, g = head-group 0..3, 4 heads each).
Each core computes its 4 output heads' attention and a partial out = y_g @ Wo_g;
host sums the 4 partials per batch.

Per-core dataflow (float32r matmuls - full PE rate, ~1.5e-4 rel rounding):
  4 phases over t-columns (512 each):
    projections into transposed layouts (qT/kT [d,t]; v natural [t,d]; lamT)
    causal attention in sT=[tk,tq] layout; ACT exp evacuates PSUM;
    denominator via ones-column matmul; normalize/combine via K=1 broadcast
    matmuls; partial output projection streamed per 512-col group.
"""
import sys
sys.path.insert(0, "/opt/trn_rl_repo")
from contextlib import ExitStack

import numpy as np

from concourse import bacc, mybir, tile
from concourse.bass_utils import run_bass_kernel_spmd

B, T, D, H = 2, 2048, 2048, 16
HPC = 4               # heads per core
NC = 8                # cores
NDC = D // 128        # 16 contraction chunks
NPH = 4               # t-phases
PT = T // NPH         # 512 t-cols per phase
SCALE = 1.0 / float(np.sqrt(D // H))

f32 = mybir.dt.float32
f32r = mybir.dt.float32r
EXP = mybir.ActivationFunctionType.Exp
SIG = mybir.ActivationFunctionType.Sigmoid

_CACHE = {}


def _build():
    nc = bacc.Bacc("TRN2", target_bir_lowering=False, debug=False)
    xT = nc.dram_tensor("xT", [D, T], f32r, kind="ExternalInput").ap()
    wq1 = nc.dram_tensor("wq1", [D, 512], f32r, kind="ExternalInput").ap()
    wq2 = nc.dram_tensor("wq2", [D, 512], f32r, kind="ExternalInput").ap()
    wk = nc.dram_tensor("wk", [D, 512], f32r, kind="ExternalInput").ap()
    wv = nc.dram_tensor("wv", [D, 512], f32r, kind="ExternalInput").ap()
    wlam = nc.dram_tensor("wlam", [D, HPC], f32r, kind="ExternalInput").ap()
    wo = nc.dram_tensor("wo", [512, D], f32r, kind="ExternalInput").ap()
    mstrip = nc.dram_tensor("mstrip", [128, 896], f32r, kind="ExternalInput").ap()
    selin = nc.dram_tensor("selin", [HPC, 512], f32r, kind="ExternalInput").ap()
    out = nc.dram_tensor("out", [T, D], f32, kind="ExternalOutput").ap()

    with tile.TileContext(nc) as tc, ExitStack() as ctx:
        ctx.enter_context(nc.allow_low_precision(reason="fp32r matmul pipeline"))
        persist = ctx.enter_context(tc.tile_pool(name="persist", bufs=1))
        xpool = ctx.enter_context(tc.tile_pool(name="xpool", bufs=1))
        qpool = ctx.enter_context(tc.tile_pool(name="qpool", bufs=1))
        wpool = ctx.enter_context(tc.tile_pool(name="wpool", bufs=2))
        epool = ctx.enter_context(tc.tile_pool(name="epool", bufs=2))
        cpool = ctx.enter_context(tc.tile_pool(name="cpool", bufs=1))
        opool = ctx.enter_context(tc.tile_pool(name="opool", bufs=2))
        # PSUM: s-pool 2bank x2 + y-class 1bank x3 + proj 1bank x1 = 8 banks
        ppmm = ctx.enter_context(tc.tile_pool(name="ppmm", bufs=2, space="PSUM"))
        ppy = ctx.enter_context(tc.tile_pool(name="ppy", bufs=3, space="PSUM"))
        pp1 = ctx.enter_context(tc.tile_pool(name="pp1", bufs=1, space="PSUM"))

        # persistent tensors
        kT = persist.tile([128, HPC, T], f32r)          # 32KB
        vn = persist.tile([128, 2, NDC, 2, 128], f32r)  # 32KB [tk,(pair,tkc,j),d]
        ms = persist.tile([128, 896], f32r)             # 3.5KB
        nc.sync.dma_start(out=ms[:], in_=mstrip[:])
        sel = persist.tile([HPC, HPC, 128], f32r)       # head-row selectors
        nc.sync.dma_start(out=sel.rearrange("p a b -> p (a b)"), in_=selin[:])
        ones_col_f = persist.tile([128, 1], f32)
        nc.vector.memset(ones_col_f[:], 1.0)
        ones_col = persist.tile([128, 1], f32r)
        nc.vector.tensor_copy(ones_col[:], ones_col_f[:])
        ones_row_f = persist.tile([1, 128], f32)
        nc.vector.memset(ones_row_f[:], 1.0)
        ones_row = persist.tile([1, 128], f32r)
        nc.vector.tensor_copy(ones_row[:], ones_row_f[:])

        for ph in range(NPH):
            t0 = PT * ph
            # ---- x^T slice for this phase: [128, dc, 512] ----
            xTh = xpool.tile([128, NDC, PT], f32r, name=f"xTh{ph}", tag="xTh")
            for dc in range(NDC):
                nc.sync.dma_start(out=xTh[:, dc],
                                  in_=xT[128 * dc:128 * (dc + 1), t0:t0 + PT])

            # ---- q projections (8 q-heads: 0..3 from wq1, 4..7 from wq2) ----
            qTh = qpool.tile([128, 8, PT], f32r, name=f"qTh{ph}", tag="qTh")
            for qh in range(8):
                wsrc = wq1 if qh < 4 else wq2
                hl = qh % 4
                wt = wpool.tile([128, NDC, 128], f32r, name=f"wq{ph}_{qh}", tag="wq")
                for dc in range(NDC):
                    nc.sync.dma_start(
                        out=wt[:, dc],
                        in_=wsrc[128 * dc:128 * (dc + 1), 128 * hl:128 * (hl + 1)])
                ps = pp1.tile([128, PT], f32, name=f"psq{ph}_{qh}", tag="p1")
                for dc in range(NDC):
                    nc.tensor.matmul(ps[:], wt[:, dc], xTh[:, dc],
                                     start=(dc == 0), stop=(dc == NDC - 1))
                nc.vector.tensor_copy(qTh[:, qh], ps[:])

            # ---- k projections (4 k-heads) ----
            for kh in range(HPC):
                wt = wpool.tile([128, NDC, 128], f32r, name=f"wk{ph}_{kh}", tag="wq")
                for dc in range(NDC):
                    nc.sync.dma_start(
                        out=wt[:, dc],
                        in_=wk[128 * dc:128 * (dc + 1), 128 * kh:128 * (kh + 1)])
                ps = pp1.tile([128, PT], f32, name=f"psk{ph}_{kh}", tag="p1")
                for dc in range(NDC):
                    nc.tensor.matmul(ps[:], wt[:, dc], xTh[:, dc],
                                     start=(dc == 0), stop=(dc == NDC - 1))
                nc.vector.tensor_copy(kT[:, kh, t0:t0 + PT], ps[:])

            # ---- v projections (2 pairs x 256 cols), natural [tk, d] layout ----
            for pair in range(2):
                wt = wpool.tile([128, NDC, 256], f32r, name=f"wv{ph}_{pair}",
                                tag="wv", bufs=1)
                for dc in range(NDC):
                    nc.sync.dma_start(
                        out=wt[:, dc],
                        in_=wv[128 * dc:128 * (dc + 1), 256 * pair:256 * (pair + 1)])
                for tsub in range(4):
                    tkc = 4 * ph + tsub
                    ps = pp1.tile([128, 256], f32, name=f"psv{ph}_{pair}_{tsub}",
                                  tag="p1")
                    for dc in range(NDC):
                        nc.tensor.matmul(
                            ps[:], xTh[:, dc, 128 * tsub:128 * (tsub + 1)],
                            wt[:, dc], start=(dc == 0), stop=(dc == NDC - 1))
                    nc.vector.tensor_copy(
                        vn[:, pair, tkc].rearrange("p a b -> p (a b)"), ps[:])

            # ---- lam projection + sigmoid ----
            wlt = wpool.tile([128, NDC, HPC], f32r, name=f"wl{ph}", tag="wl")
            for dc in range(NDC):
                nc.sync.dma_start(out=wlt[:, dc],
                                  in_=wlam[128 * dc:128 * (dc + 1), :])
            psl = ppy.tile([HPC, PT], f32, name=f"psl{ph}", tag="y")
            for dc in range(NDC):
                nc.tensor.matmul(psl[:], wlt[:, dc], xTh[:, dc],
                                 start=(dc == 0), stop=(dc == NDC - 1))
            lamS = cpool.tile([HPC, PT], f32r, name=f"lam{ph}", tag="lam", bufs=2)
            nc.scalar.activation(lamS[:], psl[:], SIG)

            # ---- attention for tq-group [t0, t0+512), 4 head-pairs ----
            ntk = 4 * (ph + 1)
            yh = qpool.tile([128, HPC, PT], f32r, name=f"yh{ph}", tag="yh")
            for hl in range(HPC):
                y_ps = {}
                rden = cpool.tile([1, 2, PT], f32r, name=f"rden{ph}_{hl}", tag="rden")
                for j, qh in enumerate((hl, 4 + hl)):
                    khl = (hl // 2) if j == 0 else (2 + hl // 2)
                    pair, pj = khl // 2, khl % 2
                    ps_y = ppy.tile([128, PT], f32, name=f"psy{ph}_{hl}_{j}", tag="y")
                    ps_den = ppy.tile([1, PT], f32, name=f"psd{ph}_{hl}_{j}",
                                      tag="y")

                    def consume(bt, ex):
                        for c in range(2):
                            tkc = 2 * bt + c
                            o = 128 * tkc - t0
                            if o >= 0:   # diagonal tile -> 0/1 mask
                                nc.vector.tensor_mul(ex[:, c], ex[:, c],
                                                     ms[:, 384 - o:896 - o])
                            nc.tensor.matmul(ps_den[:], ones_col[:], ex[:, c],
                                             start=(tkc == 0), stop=(tkc == ntk - 1))
                            nc.tensor.matmul(ps_y[:], vn[:, pair, tkc, pj], ex[:, c],
                                             start=(tkc == 0), stop=(tkc == ntk - 1))

                    prev = None
                    for bt in range(ntk // 2):
                        ps_s = ppmm.tile([128, 2, PT], f32,
                                         name=f"pss{ph}_{hl}_{j}_{bt}", tag="mm2")
                        for c in range(2):
                            tkc = 2 * bt + c
                            nc.tensor.matmul(
                                ps_s[:, c],
                                kT[:, khl, 128 * tkc:128 * (tkc + 1)],
                                qTh[:, qh], start=True, stop=True)
                        ex = epool.tile([128, 2, PT], f32r,
                                        name=f"ex{ph}_{hl}_{j}_{bt}", tag="ex")
                        nc.scalar.activation(ex[:], ps_s[:], EXP, scale=SCALE)
                        if prev is not None:
                            consume(*prev)
                        prev = (bt, ex)
                    consume(*prev)
                    y_ps[j] = ps_y
                    nc.vector.reciprocal(rden[:, j], ps_den[:])

                # combine y_h = y1*r1 - lam_h*(r2*y2)
                ps_b = ppmm.tile([128, 2, PT], f32, name=f"psb{ph}_{hl}", tag="mm2")
                nc.tensor.matmul(ps_b[:, 0], ones_row[:], rden[:, 0],
                                 start=True, stop=True)
                nc.tensor.matmul(ps_b[:, 1], ones_row[:], rden[:, 1],
                                 start=True, stop=True)
                ps_lam = pp1.tile([128, PT], f32, name=f"pslam{ph}_{hl}", tag="p1")
                nc.tensor.matmul(ps_lam[:], sel[:, hl], lamS[:],
                                 start=True, stop=True)
                rB = cpool.tile([128, 2, PT], f32, name=f"rB{ph}_{hl}", tag="rB")
                nc.vector.tensor_copy(rB[:], ps_b[:])
                t1 = cpool.tile([128, PT], f32, name=f"t1{ph}_{hl}", tag="t1")
                nc.vector.tensor_mul(t1[:], y_ps[0][:], rB[:, 0])
                t2 = cpool.tile([128, PT], f32, name=f"t2{ph}_{hl}", tag="t2")
                nc.vector.tensor_mul(t2[:], y_ps[1][:], rB[:, 1])
                nc.vector.tensor_mul(t2[:], t2[:], ps_lam[:])
                nc.vector.tensor_sub(yh[:, hl], t1[:], t2[:])

            # ---- Wo partial: out[t0:t0+512, :] = sum_h yh^T_h @ wo_h ----
            for dout in range(4):
                wo4 = wpool.tile([128, HPC, 512], f32r, name=f"wo{ph}_{dout}",
                                 tag="wo4", bufs=2)
                for hl in range(HPC):
                    nc.sync.dma_start(
                        out=wo4[:, hl],
                        in_=wo[128 * hl:128 * (hl + 1), 512 * dout:512 * (dout + 1)])
                for tsub in range(4):
                    ps_o = pp1.tile([128, 512], f32, name=f"pso{ph}_{dout}_{tsub}",
                                    tag="p1")
                    for hl in range(HPC):
                        nc.tensor.matmul(
                            ps_o[:], yh[:, hl, 128 * tsub:128 * (tsub + 1)],
                            wo4[:, hl], start=(hl == 0), stop=(hl == HPC - 1))
                    ob = opool.tile([128, 512], f32, name=f"ob{ph}_{dout}_{tsub}",
                                    tag="ob")
                    nc.vector.tensor_copy(ob[:], ps_o[:])
                    nc.sync.dma_start(
                        out=out[t0 + 128 * tsub:t0 + 128 * (tsub + 1),
                                512 * dout:512 * (dout + 1)],
                        in_=ob[:])
    nc.compile()
    return nc


def _get_nc():
    if "nc" not in _CACHE:
        _CACHE["nc"] = _build()
    return _CACHE["nc"]


def kernel(x, Wq1, Wq2, Wk, Wv, Wlam, Wo, **_ignored):
    x = np.ascontiguousarray(np.asarray(x, dtype=np.float32))
    Wq1 = np.asarray(Wq1, dtype=np.float32)
    Wq2 = np.asarray(Wq2, dtype=np.float32)
    Wk = np.asarray(Wk, dtype=np.float32)
    Wv = np.asarray(Wv, dtype=np.float32)
    Wlam = np.asarray(Wlam, dtype=np.float32)
    Wo = np.asarray(Wo, dtype=np.float32)

    cc = np.arange(896)[None, :]
    rr = np.arange(128)[:, None]
    mask = (cc >= rr + 384).astype(np.float32)
    selv = np.zeros((HPC, HPC, 128), dtype=np.float32)
    for i in range(HPC):
        selv[i, i, :] = 1.0
    selv = selv.reshape(HPC, 512)

    xTs = [np.ascontiguousarray(x[b].T) for b in range(B)]
    in_maps = []
    for core in range(NC):
        b, g = divmod(core, 4)
        kv_cols = np.r_[256 * g:256 * g + 256, 1024 + 256 * g:1024 + 256 * g + 256]
        in_maps.append({
            "xT": xTs[b],
            "wq1": np.ascontiguousarray(Wq1[:, 512 * g:512 * (g + 1)]),
            "wq2": np.ascontiguousarray(Wq2[:, 512 * g:512 * (g + 1)]),
            "wk": np.ascontiguousarray(Wk[:, kv_cols]),
            "wv": np.ascontiguousarray(Wv[:, kv_cols]),
            "wlam": np.ascontiguousarray(Wlam[:, 4 * g:4 * (g + 1)]),
            "wo": np.ascontiguousarray(Wo[512 * g:512 * (g + 1), :]),
            "mstrip": mask,
            "selin": selv,
        })

    res = run_bass_kernel_spmd(_get_nc(), in_maps, list(range(NC)), **_CACHE.get("run_kwargs", {}))
    _CACHE["last_res"] = res
    out = np.zeros((B, T, D), dtype=np.float32)
    for core in range(NC):
        out[core // 4] += res.results[core]["out"]
    return out


# revision 11
# speedup vs baseline: 1.2670x; 1.2179x over previous
"""DiffAttnV2-like fused kernel for Trainium2 (8 NeuronCores).

Sharding: core = 4*b + g  (b = batch 0..1, g = head-group 0..3, 4 heads each).
Each core computes its 4 output heads' attention and a partial out = y_g @ Wo_g;
host sums the 4 partials per batch.

Per-core dataflow (float32r matmuls - full PE rate, ~1.5e-4 rel rounding):
  4 phases over t-columns (512 each):
    projections into transposed layouts (qT/kT [d,t]; v natural [t,d]; lamT)
    causal attention in sT=[tk,tq] layout; ACT exp evacuates PSUM;
    denominator via ones-column matmul; normalize/combine via K=1 broadcast
    matmuls; partial output projection streamed per 512-col group.
"""
import sys
sys.path.insert(0, "/opt/trn_rl_repo")
from contextlib import ExitStack

import numpy as np

from concourse import bacc, mybir, tile
from concourse.bass_utils import run_bass_kernel_spmd

B, T, D, H = 2, 2048, 2048, 16
HPC = 4               # heads per core
NC = 8                # cores
NDC = D // 128        # 16 contraction chunks
NPH = 4               # t-phases
PT = T // NPH         # 512 t-cols per phase
SCALE = 1.0 / float(np.sqrt(D // H))

f32 = mybir.dt.float32
f32r = mybir.dt.float32r
EXP = mybir.ActivationFunctionType.Exp
SIG = mybir.ActivationFunctionType.Sigmoid

_CACHE = {}


def _build():
    nc = bacc.Bacc("TRN2", target_bir_lowering=False, debug=False)
    xTp = nc.dram_tensor("xTp", [NPH, 128, NDC, PT], f32r, kind="ExternalInput").ap()
    wqp = nc.dram_tensor("wqp", [8, 128, NDC, 128], f32r, kind="ExternalInput").ap()
    wkp = nc.dram_tensor("wkp", [HPC, 128, NDC, 128], f32r, kind="ExternalInput").ap()
    wvp = nc.dram_tensor("wvp", [2, 128, NDC, 256], f32r, kind="ExternalInput").ap()
    wlamp = nc.dram_tensor("wlamp", [128, NDC, HPC], f32r, kind="ExternalInput").ap()
    wop = nc.dram_tensor("wop", [4, 128, HPC, 512], f32r, kind="ExternalInput").ap()
    mstrip = nc.dram_tensor("mstrip", [128, 896], f32r, kind="ExternalInput").ap()
    selin = nc.dram_tensor("selin", [HPC, 512], f32r, kind="ExternalInput").ap()
    out = nc.dram_tensor("out", [T, D], f32, kind="ExternalOutput").ap()

    with tile.TileContext(nc) as tc, ExitStack() as ctx:
        ctx.enter_context(nc.allow_low_precision(reason="fp32r matmul pipeline"))
        persist = ctx.enter_context(tc.tile_pool(name="persist", bufs=1))
        xpool = ctx.enter_context(tc.tile_pool(name="xpool", bufs=1))
        qpool = ctx.enter_context(tc.tile_pool(name="qpool", bufs=1))
        wpool = ctx.enter_context(tc.tile_pool(name="wpool", bufs=2))
        epool = ctx.enter_context(tc.tile_pool(name="epool", bufs=2))
        cpool = ctx.enter_context(tc.tile_pool(name="cpool", bufs=1))
        opool = ctx.enter_context(tc.tile_pool(name="opool", bufs=2))
        # PSUM: s-pool 2bank x2 + y-class 1bank x3 + proj 1bank x1 = 8 banks
        ppmm = ctx.enter_context(tc.tile_pool(name="ppmm", bufs=2, space="PSUM"))
        ppy = ctx.enter_context(tc.tile_pool(name="ppy", bufs=3, space="PSUM"))
        pp1 = ctx.enter_context(tc.tile_pool(name="pp1", bufs=1, space="PSUM"))

        # persistent tensors
        kT = persist.tile([128, HPC, T], f32r)          # 32KB
        vn = persist.tile([128, 2, NDC, 2, 128], f32r)  # 32KB [tk,(pair,tkc,j),d]
        ms = persist.tile([128, 896], f32r)             # 3.5KB
        nc.sync.dma_start(out=ms[:], in_=mstrip[:])
        sel = persist.tile([HPC, HPC, 128], f32r)       # head-row selectors
        nc.sync.dma_start(out=sel.rearrange("p a b -> p (a b)"), in_=selin[:])
        ones_col_f = persist.tile([128, 1], f32)
        nc.vector.memset(ones_col_f[:], 1.0)
        ones_col = persist.tile([128, 1], f32r)
        nc.vector.tensor_copy(ones_col[:], ones_col_f[:])
        ones_row_f = persist.tile([1, 128], f32)
        nc.vector.memset(ones_row_f[:], 1.0)
        ones_row = persist.tile([1, 128], f32r)
        nc.vector.tensor_copy(ones_row[:], ones_row_f[:])

        for ph in range(NPH):
            t0 = PT * ph
            # ---- x^T slice for this phase: [128, dc, 512] ----
            xTh = xpool.tile([128, NDC, PT], f32r, name=f"xTh{ph}", tag="xTh")
            nc.sync.dma_start(out=xTh[:], in_=xTp[ph])

            # ---- q projections (8 q-heads: 0..3 from wq1, 4..7 from wq2) ----
            qTh = qpool.tile([128, 8, PT], f32r, name=f"qTh{ph}", tag="qTh")
            for qh in range(8):
                wt = wpool.tile([128, NDC, 128], f32r, name=f"wq{ph}_{qh}", tag="wq")
                nc.sync.dma_start(out=wt[:], in_=wqp[qh])
                ps = pp1.tile([128, PT], f32, name=f"psq{ph}_{qh}", tag="p1")
                for dc in range(NDC):
                    nc.tensor.matmul(ps[:], wt[:, dc], xTh[:, dc],
                                     start=(dc == 0), stop=(dc == NDC - 1))
                nc.vector.tensor_copy(qTh[:, qh], ps[:])

            # ---- k projections (4 k-heads) ----
            for kh in range(HPC):
                wt = wpool.tile([128, NDC, 128], f32r, name=f"wk{ph}_{kh}", tag="wq")
                nc.sync.dma_start(out=wt[:], in_=wkp[kh])
                ps = pp1.tile([128, PT], f32, name=f"psk{ph}_{kh}", tag="p1")
                for dc in range(NDC):
                    nc.tensor.matmul(ps[:], wt[:, dc], xTh[:, dc],
                                     start=(dc == 0), stop=(dc == NDC - 1))
                nc.vector.tensor_copy(kT[:, kh, t0:t0 + PT], ps[:])

            # ---- v projections (2 pairs x 256 cols), natural [tk, d] layout ----
            for pair in range(2):
                wt = wpool.tile([128, NDC, 256], f32r, name=f"wv{ph}_{pair}",
                                tag="wv", bufs=1)
                nc.sync.dma_start(out=wt[:], in_=wvp[pair])
                for tsub in range(4):
                    tkc = 4 * ph + tsub
                    ps = pp1.tile([128, 256], f32, name=f"psv{ph}_{pair}_{tsub}",
                                  tag="p1")
                    for dc in range(NDC):
                        nc.tensor.matmul(
                            ps[:], xTh[:, dc, 128 * tsub:128 * (tsub + 1)],
                            wt[:, dc], start=(dc == 0), stop=(dc == NDC - 1))
                    nc.vector.tensor_copy(
                        vn[:, pair, tkc].rearrange("p a b -> p (a b)"), ps[:])

            # ---- lam projection + sigmoid ----
            wlt = wpool.tile([128, NDC, HPC], f32r, name=f"wl{ph}", tag="wl")
            nc.sync.dma_start(out=wlt[:], in_=wlamp[:])
            psl = ppy.tile([HPC, PT], f32, name=f"psl{ph}", tag="y")
            for dc in range(NDC):
                nc.tensor.matmul(psl[:], wlt[:, dc], xTh[:, dc],
                                 start=(dc == 0), stop=(dc == NDC - 1))
            lamS = cpool.tile([HPC, PT], f32r, name=f"lam{ph}", tag="lam", bufs=2)
            nc.scalar.activation(lamS[:], psl[:], SIG)

            # ---- attention for tq-group [t0, t0+512), 4 head-pairs ----
            ntk = 4 * (ph + 1)
            yh = qpool.tile([128, HPC, PT], f32r, name=f"yh{ph}", tag="yh")
            for hl in range(HPC):
                y_ps = {}
                rden = cpool.tile([1, 2, PT], f32r, name=f"rden{ph}_{hl}", tag="rden")
                for j, qh in enumerate((hl, 4 + hl)):
                    khl = (hl // 2) if j == 0 else (2 + hl // 2)
                    pair, pj = khl // 2, khl % 2
                    ps_y = ppy.tile([128, PT], f32, name=f"psy{ph}_{hl}_{j}", tag="y")
                    ps_den = ppy.tile([1, PT], f32, name=f"psd{ph}_{hl}_{j}",
                                      tag="y")

                    def consume(bt, ex):
                        for c in range(2):
                            tkc = 2 * bt + c
                            o = 128 * tkc - t0
                            if o >= 0:   # diagonal tile -> 0/1 mask
                                nc.vector.tensor_mul(ex[:, c], ex[:, c],
                                                     ms[:, 384 - o:896 - o])
                            nc.tensor.matmul(ps_den[:], ones_col[:], ex[:, c],
                                             start=(tkc == 0), stop=(tkc == ntk - 1))
                            nc.tensor.matmul(ps_y[:], vn[:, pair, tkc, pj], ex[:, c],
                                             start=(tkc == 0), stop=(tkc == ntk - 1))

                    prev = None
                    for bt in range(ntk // 2):
                        ps_s = ppmm.tile([128, 2, PT], f32,
                                         name=f"pss{ph}_{hl}_{j}_{bt}", tag="mm2")
                        for c in range(2):
                            tkc = 2 * bt + c
                            nc.tensor.matmul(
                                ps_s[:, c],
                                kT[:, khl, 128 * tkc:128 * (tkc + 1)],
                                qTh[:, qh], start=True, stop=True)
                        ex = epool.tile([128, 2, PT], f32r,
                                        name=f"ex{ph}_{hl}_{j}_{bt}", tag="ex")
                        nc.scalar.activation(ex[:], ps_s[:], EXP, scale=SCALE)
                        if prev is not None:
                            consume(*prev)
                        prev = (bt, ex)
                    consume(*prev)
                    y_ps[j] = ps_y
                    nc.vector.reciprocal(rden[:, j], ps_den[:])

                # combine y_h = y1*r1 - lam_h*(r2*y2)
                ps_b = ppmm.tile([128, 2, PT], f32, name=f"psb{ph}_{hl}", tag="mm2")
                nc.tensor.matmul(ps_b[:, 0], ones_row[:], rden[:, 0],
                                 start=True, stop=True)
                nc.tensor.matmul(ps_b[:, 1], ones_row[:], rden[:, 1],
                                 start=True, stop=True)
                ps_lam = pp1.tile([128, PT], f32, name=f"pslam{ph}_{hl}", tag="p1")
                nc.tensor.matmul(ps_lam[:], sel[:, hl], lamS[:],
                                 start=True, stop=True)
                rB = cpool.tile([128, 2, PT], f32, name=f"rB{ph}_{hl}", tag="rB")
                nc.vector.tensor_copy(rB[:], ps_b[:])
                t1 = cpool.tile([128, PT], f32, name=f"t1{ph}_{hl}", tag="t1")
                nc.vector.tensor_mul(t1[:], y_ps[0][:], rB[:, 0])
                t2 = cpool.tile([128, PT], f32, name=f"t2{ph}_{hl}", tag="t2")
                nc.vector.tensor_mul(t2[:], y_ps[1][:], rB[:, 1])
                nc.vector.tensor_mul(t2[:], t2[:], ps_lam[:])
                nc.vector.tensor_sub(yh[:, hl], t1[:], t2[:])

            # ---- Wo partial: out[t0:t0+512, :] = sum_h yh^T_h @ wo_h ----
            for dout in range(4):
                wo4 = wpool.tile([128, HPC, 512], f32r, name=f"wo{ph}_{dout}",
                                 tag="wo4", bufs=1)
                nc.sync.dma_start(out=wo4[:], in_=wop[dout])
                for tsub in range(4):
                    ps_o = pp1.tile([128, 512], f32, name=f"pso{ph}_{dout}_{tsub}",
                                    tag="p1")
                    for hl in range(HPC):
                        nc.tensor.matmul(
                            ps_o[:], yh[:, hl, 128 * tsub:128 * (tsub + 1)],
                            wo4[:, hl], start=(hl == 0), stop=(hl == HPC - 1))
                    ob = opool.tile([128, 512], f32, name=f"ob{ph}_{dout}_{tsub}",
                                    tag="ob")
                    nc.vector.tensor_copy(ob[:], ps_o[:])
                    nc.sync.dma_start(
                        out=out[t0 + 128 * tsub:t0 + 128 * (tsub + 1),
                                512 * dout:512 * (dout + 1)],
                        in_=ob[:])
    nc.compile()
    return nc


def _get_nc():
    if "nc" not in _CACHE:
        _CACHE["nc"] = _build()
    return _CACHE["nc"]


def kernel(x, Wq1, Wq2, Wk, Wv, Wlam, Wo, **_ignored):
    x = np.ascontiguousarray(np.asarray(x, dtype=np.float32))
    Wq1 = np.asarray(Wq1, dtype=np.float32)
    Wq2 = np.asarray(Wq2, dtype=np.float32)
    Wk = np.asarray(Wk, dtype=np.float32)
    Wv = np.asarray(Wv, dtype=np.float32)
    Wlam = np.asarray(Wlam, dtype=np.float32)
    Wo = np.asarray(Wo, dtype=np.float32)

    cc = np.arange(896)[None, :]
    rr = np.arange(128)[:, None]
    mask = (cc >= rr + 384).astype(np.float32)
    selv = np.zeros((HPC, HPC, 128), dtype=np.float32)
    for i in range(HPC):
        selv[i, i, :] = 1.0
    selv = selv.reshape(HPC, 512)

    def chunk_cols(w):
        # [D, C] -> [C//128 heads? no: generic [D, C] -> [C/128? ] ] handled per-use
        return w

    xTs = []
    for b in range(B):
        xt = x[b].T                                   # [D, T]
        xTs.append(np.ascontiguousarray(
            xt.reshape(NDC, 128, NPH, PT).transpose(2, 1, 0, 3)))

    in_maps = []
    for core in range(NC):
        b, g = divmod(core, 4)
        kv_cols = np.r_[256 * g:256 * g + 256, 1024 + 256 * g:1024 + 256 * g + 256]
        wq_s = np.concatenate([Wq1[:, 512 * g:512 * (g + 1)],
                               Wq2[:, 512 * g:512 * (g + 1)]], axis=1)  # [D, 1024]
        wqp_v = np.ascontiguousarray(
            wq_s.reshape(NDC, 128, 8, 128).transpose(2, 1, 0, 3))
        wk_s = Wk[:, kv_cols]
        wkp_v = np.ascontiguousarray(
            wk_s.reshape(NDC, 128, HPC, 128).transpose(2, 1, 0, 3))
        wv_s = Wv[:, kv_cols]
        wvp_v = np.ascontiguousarray(
            wv_s.reshape(NDC, 128, 2, 256).transpose(2, 1, 0, 3))
        wlam_s = Wlam[:, 4 * g:4 * (g + 1)]
        wlamp_v = np.ascontiguousarray(
            wlam_s.reshape(NDC, 128, HPC).transpose(1, 0, 2))
        wo_s = Wo[512 * g:512 * (g + 1), :]
        wop_v = np.ascontiguousarray(
            wo_s.reshape(HPC, 128, 4, 512).transpose(2, 1, 0, 3))
        in_maps.append({
            "xTp": xTs[b],
            "wqp": wqp_v,
            "wkp": wkp_v,
            "wvp": wvp_v,
            "wlamp": wlamp_v,
            "wop": wop_v,
            "mstrip": mask,
            "selin": selv,
        })

    res = run_bass_kernel_spmd(_get_nc(), in_maps, list(range(NC)), **_CACHE.get("run_kwargs", {}))
    _CACHE["last_res"] = res
    out = np.zeros((B, T, D), dtype=np.float32)
    for core in range(NC):
        out[core // 4] += res.results[core]["out"]
    return out


# revision 12
# speedup vs baseline: 1.4544x; 1.1479x over previous
"""DiffAttnV2-like fused kernel for Trainium2 (8 NeuronCores).

Sharding: core = 4*b + g  (b = batch 0..1, g = head-group 0..3, 4 heads each).
Each core computes its 4 output heads' attention and a partial out = y_g @ Wo_g;
host sums the 4 partials per batch.

Per-core dataflow (float32r matmuls - full PE rate, ~1.5e-4 rel rounding):
  4 phases over t-columns (512 each):
    projections into transposed layouts (qT/kT [d,t]; v natural [t,d]; lamT)
    causal attention in sT=[tk,tq] layout; ACT exp evacuates PSUM;
    denominator via ones-column matmul; normalize/combine via K=1 broadcast
    matmuls; partial output projection streamed per 512-col group.
"""
import sys
sys.path.insert(0, "/opt/trn_rl_repo")
from contextlib import ExitStack

import numpy as np

from concourse import bacc, mybir, tile
from concourse.bass_utils import run_bass_kernel_spmd

B, T, D, H = 2, 2048, 2048, 16
HPC = 4               # heads per core
NC = 8                # cores
NDC = D // 128        # 16 contraction chunks
NPH = 4               # t-phases
PT = T // NPH         # 512 t-cols per phase
SCALE = 1.0 / float(np.sqrt(D // H))

f32 = mybir.dt.float32
f32r = mybir.dt.float32r
EXP = mybir.ActivationFunctionType.Exp
SIG = mybir.ActivationFunctionType.Sigmoid

_CACHE = {}


def _build():
    nc = bacc.Bacc("TRN2", target_bir_lowering=False, debug=False)
    xTp = nc.dram_tensor("xTp", [NPH, 128, NDC, PT], f32r, kind="ExternalInput").ap()
    wqp = nc.dram_tensor("wqp", [8, 128, NDC, 128], f32r, kind="ExternalInput").ap()
    wkp = nc.dram_tensor("wkp", [HPC, 128, NDC, 128], f32r, kind="ExternalInput").ap()
    wvp = nc.dram_tensor("wvp", [2, 128, NDC, 256], f32r, kind="ExternalInput").ap()
    wlamp = nc.dram_tensor("wlamp", [128, NDC, HPC], f32r, kind="ExternalInput").ap()
    wop = nc.dram_tensor("wop", [4, 128, HPC, 512], f32r, kind="ExternalInput").ap()
    mstrip = nc.dram_tensor("mstrip", [128, 896], f32r, kind="ExternalInput").ap()
    selin = nc.dram_tensor("selin", [HPC, 512], f32r, kind="ExternalInput").ap()
    out = nc.dram_tensor("out", [T, D], f32, kind="ExternalOutput").ap()

    with tile.TileContext(nc) as tc, ExitStack() as ctx:
        ctx.enter_context(nc.allow_low_precision(reason="fp32r matmul pipeline"))
        persist = ctx.enter_context(tc.tile_pool(name="persist", bufs=1))
        xpool = ctx.enter_context(tc.tile_pool(name="xpool", bufs=1))
        qpool = ctx.enter_context(tc.tile_pool(name="qpool", bufs=1))
        wpool = ctx.enter_context(tc.tile_pool(name="wpool", bufs=2))
        epool = ctx.enter_context(tc.tile_pool(name="epool", bufs=2))
        cpool = ctx.enter_context(tc.tile_pool(name="cpool", bufs=1))
        opool = ctx.enter_context(tc.tile_pool(name="opool", bufs=2))
        # PSUM: s4 (4 banks x1) + acc (1x1) + den (1x1) + tr (1x2) = 8 banks
        pps = ctx.enter_context(tc.tile_pool(name="pps", bufs=1, space="PSUM"))
        ppacc = ctx.enter_context(tc.tile_pool(name="ppacc", bufs=1, space="PSUM"))
        ppden = ctx.enter_context(tc.tile_pool(name="ppden", bufs=1, space="PSUM"))
        pptr = ctx.enter_context(tc.tile_pool(name="pptr", bufs=2, space="PSUM"))

        # persistent tensors
        kT = persist.tile([128, HPC, T], f32r)          # 32KB
        vn = persist.tile([128, 2, NDC, 2, 128], f32r)  # 32KB [tk,(pair,tkc,j),d]
        ms = persist.tile([128, 896], f32r)             # 3.5KB
        nc.sync.dma_start(out=ms[:], in_=mstrip[:])
        sel = persist.tile([HPC, HPC, 128], f32r)       # head-row selectors
        nc.sync.dma_start(out=sel.rearrange("p a b -> p (a b)"), in_=selin[:])
        ones_col_f = persist.tile([128, 1], f32)
        nc.vector.memset(ones_col_f[:], 1.0)
        ones_col = persist.tile([128, 1], f32r)
        nc.vector.tensor_copy(ones_col[:], ones_col_f[:])
        ones_row_f = persist.tile([1, 128], f32)
        nc.vector.memset(ones_row_f[:], 1.0)
        ones_row = persist.tile([1, 128], f32r)
        nc.vector.tensor_copy(ones_row[:], ones_row_f[:])

        for ph in range(NPH):
            t0 = PT * ph
            # ---- x^T slice for this phase: [128, dc, 512] ----
            xTh = xpool.tile([128, NDC, PT], f32r, name=f"xTh{ph}", tag="xTh")
            nc.sync.dma_start(out=xTh[:], in_=xTp[ph])

            # ---- q projections (8 q-heads: 0..3 from wq1, 4..7 from wq2) ----
            qTh = qpool.tile([128, 8, PT], f32r, name=f"qTh{ph}", tag="qTh")
            for qh in range(8):
                wt = wpool.tile([128, NDC, 128], f32r, name=f"wq{ph}_{qh}", tag="wq")
                nc.sync.dma_start(out=wt[:], in_=wqp[qh])
                ps = pptr.tile([128, PT], f32, name=f"psq{ph}_{qh}", tag="tr")
                for dc in range(NDC):
                    nc.tensor.matmul(ps[:], wt[:, dc], xTh[:, dc],
                                     start=(dc == 0), stop=(dc == NDC - 1))
                nc.vector.tensor_copy(qTh[:, qh], ps[:])

            # ---- k projections (4 k-heads) ----
            for kh in range(HPC):
                wt = wpool.tile([128, NDC, 128], f32r, name=f"wk{ph}_{kh}", tag="wq")
                nc.sync.dma_start(out=wt[:], in_=wkp[kh])
                ps = pptr.tile([128, PT], f32, name=f"psk{ph}_{kh}", tag="tr")
                for dc in range(NDC):
                    nc.tensor.matmul(ps[:], wt[:, dc], xTh[:, dc],
                                     start=(dc == 0), stop=(dc == NDC - 1))
                nc.vector.tensor_copy(kT[:, kh, t0:t0 + PT], ps[:])

            # ---- v projections (2 pairs x 256 cols), natural [tk, d] layout ----
            for pair in range(2):
                wt = wpool.tile([128, NDC, 256], f32r, name=f"wv{ph}_{pair}",
                                tag="wv", bufs=1)
                nc.sync.dma_start(out=wt[:], in_=wvp[pair])
                for tsub in range(4):
                    tkc = 4 * ph + tsub
                    ps = pptr.tile([128, 256], f32, name=f"psv{ph}_{pair}_{tsub}",
                                   tag="tr")
                    for dc in range(NDC):
                        nc.tensor.matmul(
                            ps[:], xTh[:, dc, 128 * tsub:128 * (tsub + 1)],
                            wt[:, dc], start=(dc == 0), stop=(dc == NDC - 1))
                    nc.vector.tensor_copy(
                        vn[:, pair, tkc].rearrange("p a b -> p (a b)"), ps[:])

            # ---- lam projection + sigmoid ----
            wlt = wpool.tile([128, NDC, HPC], f32r, name=f"wl{ph}", tag="wl")
            nc.sync.dma_start(out=wlt[:], in_=wlamp[:])
            psl = pptr.tile([HPC, PT], f32, name=f"psl{ph}", tag="tr")
            for dc in range(NDC):
                nc.tensor.matmul(psl[:], wlt[:, dc], xTh[:, dc],
                                 start=(dc == 0), stop=(dc == NDC - 1))
            lamS = cpool.tile([HPC, PT], f32r, name=f"lam{ph}", tag="lam", bufs=2)
            nc.scalar.activation(lamS[:], psl[:], SIG)

            # ---- attention for tq-group [t0, t0+512), 4 head-pairs ----
            ntk = 4 * (ph + 1)
            nbt = ntk // 4
            yh = qpool.tile([128, HPC, PT], f32r, name=f"yh{ph}", tag="yh")
            for hl in range(HPC):
                t1 = None
                pending = None   # closure: finish j0 combine after j1 starts
                for j, qh in enumerate((hl, 4 + hl)):
                    khl = (hl // 2) if j == 0 else (2 + hl // 2)
                    pair, pj = khl // 2, khl % 2
                    ps_y = ppacc.tile([128, PT], f32, name=f"psy{ph}_{hl}_{j}",
                                      tag="acc")
                    ps_den = ppden.tile([1, PT], f32, name=f"psd{ph}_{hl}_{j}",
                                        tag="den")

                    def consume(bt, ex, ps_y=ps_y, ps_den=ps_den, pair=pair, pj=pj):
                        for c in range(4):
                            tkc = 4 * bt + c
                            o = 128 * tkc - t0
                            if o >= 0:   # diagonal tile -> 0/1 mask
                                nc.vector.tensor_mul(ex[:, c], ex[:, c],
                                                     ms[:, 384 - o:896 - o])
                            nc.tensor.matmul(ps_den[:], ones_col[:], ex[:, c],
                                             start=(tkc == 0), stop=(tkc == ntk - 1))
                            nc.tensor.matmul(ps_y[:], vn[:, pair, tkc, pj], ex[:, c],
                                             start=(tkc == 0), stop=(tkc == ntk - 1))

                    prev = None
                    for bt in range(nbt):
                        ps_s = pps.tile([128, 4, PT], f32,
                                        name=f"pss{ph}_{hl}_{j}_{bt}", tag="s4")
                        for c in range(4):
                            tkc = 4 * bt + c
                            nc.tensor.matmul(
                                ps_s[:, c],
                                kT[:, khl, 128 * tkc:128 * (tkc + 1)],
                                qTh[:, qh], start=True, stop=True)
                        ex = epool.tile([128, 4, PT], f32r,
                                        name=f"ex{ph}_{hl}_{j}_{bt}", tag="ex")
                        nc.scalar.activation(ex[:], ps_s[:], EXP, scale=SCALE)
                        if bt == 0 and pending is not None:
                            pending()   # j0's bcast+normalize overlaps j1 start
                            pending = None
                        if prev is not None:
                            consume(*prev)
                        prev = (bt, ex)
                    consume(*prev)

                    # reciprocal of denominator (frees den bank)
                    rd_f = cpool.tile([1, PT], f32, name=f"rdf{ph}_{hl}_{j}",
                                      tag="rdf")
                    nc.vector.reciprocal_approx_fast(rd_f[:], ps_den[:])
                    rden_j = cpool.tile([1, PT], f32r, name=f"rden{ph}_{hl}_{j}",
                                        tag=f"rden{j}")
                    nc.vector.tensor_copy(rden_j[:], rd_f[:])

                    def combine_j(j=j, ps_y=ps_y, rden_j=rden_j, hl=hl):
                        nonlocal t1
                        ps_b = pptr.tile([128, PT], f32, name=f"psb{ph}_{hl}_{j}",
                                         tag="tr")
                        nc.tensor.matmul(ps_b[:], ones_row[:], rden_j[:],
                                         start=True, stop=True)
                        rB = cpool.tile([128, PT], f32, name=f"rB{ph}_{hl}_{j}",
                                        tag="rB")
                        nc.vector.tensor_copy(rB[:], ps_b[:])
                        if j == 0:
                            t1 = cpool.tile([128, PT], f32, name=f"t1{ph}_{hl}",
                                            tag="t1")
                            nc.vector.tensor_mul(t1[:], ps_y[:], rB[:])
                        else:
                            ps_lam = pptr.tile([128, PT], f32,
                                               name=f"pslam{ph}_{hl}", tag="tr")
                            nc.tensor.matmul(ps_lam[:], sel[:, hl], lamS[:],
                                             start=True, stop=True)
                            t2 = cpool.tile([128, PT], f32, name=f"t2{ph}_{hl}",
                                            tag="t2")
                            nc.vector.tensor_mul(t2[:], ps_y[:], rB[:])
                            nc.vector.tensor_mul(t2[:], t2[:], ps_lam[:])
                            nc.vector.tensor_sub(yh[:, hl], t1[:], t2[:])

                    if j == 0:
                        pending = combine_j
                    else:
                        combine_j()

            # ---- Wo partial: out[t0:t0+512, :] = sum_h yh^T_h @ wo_h ----
            for dout in range(4):
                wo4 = wpool.tile([128, HPC, 512], f32r, name=f"wo{ph}_{dout}",
                                 tag="wo4", bufs=1)
                nc.sync.dma_start(out=wo4[:], in_=wop[dout])
                for tsub in range(4):
                    ps_o = pptr.tile([128, 512], f32, name=f"pso{ph}_{dout}_{tsub}",
                                     tag="tr")
                    for hl in range(HPC):
                        nc.tensor.matmul(
                            ps_o[:], yh[:, hl, 128 * tsub:128 * (tsub + 1)],
                            wo4[:, hl], start=(hl == 0), stop=(hl == HPC - 1))
                    ob = opool.tile([128, 512], f32, name=f"ob{ph}_{dout}_{tsub}",
                                    tag="ob")
                    nc.vector.tensor_copy(ob[:], ps_o[:])
                    nc.sync.dma_start(
                        out=out[t0 + 128 * tsub:t0 + 128 * (tsub + 1),
                                512 * dout:512 * (dout + 1)],
                        in_=ob[:])
    nc.compile()
    return nc


def _get_nc():
    if "nc" not in _CACHE:
        _CACHE["nc"] = _build()
    return _CACHE["nc"]


def kernel(x, Wq1, Wq2, Wk, Wv, Wlam, Wo, **_ignored):
    x = np.ascontiguousarray(np.asarray(x, dtype=np.float32))
    Wq1 = np.asarray(Wq1, dtype=np.float32)
    Wq2 = np.asarray(Wq2, dtype=np.float32)
    Wk = np.asarray(Wk, dtype=np.float32)
    Wv = np.asarray(Wv, dtype=np.float32)
    Wlam = np.asarray(Wlam, dtype=np.float32)
    Wo = np.asarray(Wo, dtype=np.float32)

    cc = np.arange(896)[None, :]
    rr = np.arange(128)[:, None]
    mask = (cc >= rr + 384).astype(np.float32)
    selv = np.zeros((HPC, HPC, 128), dtype=np.float32)
    for i in range(HPC):
        selv[i, i, :] = 1.0
    selv = selv.reshape(HPC, 512)

    def chunk_cols(w):
        # [D, C] -> [C//128 heads? no: generic [D, C] -> [C/128? ] ] handled per-use
        return w

    xTs = []
    for b in range(B):
        xt = x[b].T                                   # [D, T]
        xTs.append(np.ascontiguousarray(
            xt.reshape(NDC, 128, NPH, PT).transpose(2, 1, 0, 3)))

    in_maps = []
    for core in range(NC):
        b, g = divmod(core, 4)
        kv_cols = np.r_[256 * g:256 * g + 256, 1024 + 256 * g:1024 + 256 * g + 256]
        wq_s = np.concatenate([Wq1[:, 512 * g:512 * (g + 1)],
                               Wq2[:, 512 * g:512 * (g + 1)]], axis=1)  # [D, 1024]
        wqp_v = np.ascontiguousarray(
            wq_s.reshape(NDC, 128, 8, 128).transpose(2, 1, 0, 3))
        wk_s = Wk[:, kv_cols]
        wkp_v = np.ascontiguousarray(
            wk_s.reshape(NDC, 128, HPC, 128).transpose(2, 1, 0, 3))
        wv_s = Wv[:, kv_cols]
        wvp_v = np.ascontiguousarray(
            wv_s.reshape(NDC, 128, 2, 256).transpose(2, 1, 0, 3))
        wlam_s = Wlam[:, 4 * g:4 * (g + 1)]
        wlamp_v = np.ascontiguousarray(
            wlam_s.reshape(NDC, 128, HPC).transpose(1, 0, 2))
        wo_s = Wo[512 * g:512 * (g + 1), :]
        wop_v = np.ascontiguousarray(
            wo_s.reshape(HPC, 128, 4, 512).transpose(2, 1, 0, 3))
        in_maps.append({
            "xTp": xTs[b],
            "wqp": wqp_v,
            "wkp": wkp_v,
            "wvp": wvp_v,
            "wlamp": wlamp_v,
            "wop": wop_v,
            "mstrip": mask,
            "selin": selv,
        })

    res = run_bass_kernel_spmd(_get_nc(), in_maps, list(range(NC)), **_CACHE.get("run_kwargs", {}))
    _CACHE["last_res"] = res
    out = np.zeros((B, T, D), dtype=np.float32)
    for core in range(NC):
        out[core // 4] += res.results[core]["out"]
    return out
